# revision 8
# baseline (speedup 1.0000x reference)
# Trainium2 Bass kernel for nn_Conv_58394375356438:
# single-input-channel 7x7 conv, B=32, F=64, H=W=224, fp32.
#
# Data-parallel across 8 NeuronCores (4 images each). Per core, a
# "block-im2col" scheme: the host pre-builds a shift-replicated layout
# xpr[b, q=(7*rho+dj), P=row/2, c] (7 horizontal shifts x 4-row
# phases); each 16-row group tile then loads with ONE clean DMA per
# image into a [28, 4*448] SBUF tile (4 row-blocks along the free dim,
# 2 images packed). Output rows are computed in pairs (r, r+half): one
# PSUM accumulation chain of 3-4 K=28 fp32r matmuls (M=128 = 64
# channels x 2 rows) at tile_position (0,0). Weight tables are
# host-precomputed and loaded in one DMA. PSUM is drained by
# VectorE/ScalarE into staging tiles shaped so each output DMA writes
# 128 partitions with 4.5KB contiguous HBM runs. Output DMAs issue on
# the ACT HWDGE ring, build DMAs on the SP ring.
import sys

sys.path.insert(0, "/opt/trn_rl_repo")

import numpy as np

import concourse.bacc as bacc
import concourse.bass as bass
import concourse.mybir as mybir
import concourse.tile as tile
from concourse.bass_utils import run_bass_kernel_spmd

N_CORES = 8
B_FULL, F, KS, PAD = 32, 64, 7, 3
H = W = 224
B_LOC = B_FULL // N_CORES          # images per core
B2 = 2                             # images packed per matmul N
GROUP = 10                         # output rows per group
TILE_ROWS = 16
N_GROUPS = (H + GROUP - 1) // GROUP
NP = 118                           # even-row block positions: rows 0,2,...,234
NQ = 28                            # 7 shifts x 4 row phases

f32 = mybir.dt.float32
f32r = mybir.dt.float32r
bf16 = mybir.dt.bfloat16

MODE = "f32r"                      # "f32r" | "bf16"

# ordered weight tables: (spacing, t0); table row (7*rho+dj), col f+64*s =
# kernel[f, t0 - spacing*s + rho, dj] (0 outside range)
TABLES = [(5, t) for t in range(-3, 12)] + [(2, t) for t in (-1, 0, 3, 4, 7, 8)]
TIDX = {st: i for i, st in enumerate(TABLES)}
NT = len(TABLES)

_cache = {}


def _make_weight_tables(kern):
    # [28, NT, 128] so the device can load all tables in one DMA
    wtab = np.zeros((NQ, NT, 128), np.float32)
    for i, (spacing, t0) in enumerate(TABLES):
        for s in range(2):
            t = t0 - spacing * s
            for rho in range(4):
                di = t + rho
                if 0 <= di < KS:
                    wtab[7 * rho:7 * rho + KS, i, 64 * s:64 * s + F] = kern[:, di, :].T
    return wtab


def _make_xpr(input):
    # xpr[b, 7*rho+dj, P, c] = xpad[b, 2P + rho, c + dj]
    xpad = np.zeros((B_FULL, 240, W + 2 * PAD), np.float32)
    xpad[:, PAD:PAD + H, PAD:PAD + W] = input[:, 0]
    xpr = np.empty((B_FULL, NQ, NP, W), np.float32)
    for rho in range(4):
        for dj in range(KS):
            xpr[:, 7 * rho + dj, :, :] = xpad[:, rho:rho + 2 * NP:2, dj:dj + W]
    return xpr


def _build_program(mode):
    mmdt = {"f32r": f32r, "bf16": bf16, "f32": f32}[mode]

    nc = bacc.Bacc("TRN2", target_bir_lowering=False)

    xpr = nc.dram_tensor("xpr", [B_LOC, NQ, NP, W], mmdt, kind="ExternalInput")
    wtab_d = nc.dram_tensor("wtab", [NQ, NT, 128], mmdt, kind="ExternalInput")
    out_d = nc.dram_tensor("out", [B_LOC, F, H, W], f32, kind="ExternalOutput")

    with tile.TileContext(nc) as tc:
        with tc.tile_pool(name="wt", bufs=1) as wtpool, \
             tc.tile_pool(name="bt", bufs=4) as btpool, \
             tc.tile_pool(name="stage", bufs=4) as stpool, \
             tc.tile_pool(name="psum", bufs=7, space="PSUM") as pspool:

            wtall = wtpool.tile([NQ, NT * 128], mmdt)
            nc.sync.dma_start(
                wtall[:].rearrange("q (i f) -> q i f", i=NT), wtab_d[:])

            for pair in range(B_LOC // B2):
                b0 = B2 * pair
                for q in range(N_GROUPS):
                    g = GROUP * q
                    nrows = min(GROUP, H - g)
                    half = nrows // 2
                    # bt free layout: blk*448 + bi*224 + c
                    bt = btpool.tile([NQ, 4 * B2 * W], mmdt, tag="bt")
                    bt4 = bt[:].rearrange("q (k b c) -> q k b c", k=4, b=B2)
                    for bi in range(B2):
                        nc.sync.dma_start(
                            bt4[:, :, bi, :],
                            xpr[b0 + bi, :, g // 2:g // 2 + 8:2, :])
                    stA = stpool.tile([128, half * W], f32, tag="stA")
                    stB = stpool.tile([128, half * W], f32, tag="stB")
                    for p in range(half):
                        ps = pspool.tile([128, B2 * W], f32, tag="ps")
                        blks = list(range(p // 4, min(3, (p + half + 6) // 4) + 1))
                        for ib, blk in enumerate(blks):
                            i = TIDX[(half, 4 * blk - p)]
                            nc.tensor.matmul(
                                ps[:],
                                wtall[:, i * 128:(i + 1) * 128],
                                bt[:, blk * B2 * W:(blk + 1) * B2 * W],
                                start=(ib == 0), stop=(ib == len(blks) - 1),
                                tile_position=(0, 0))
                        nc.vector.tensor_copy(stA[:, p * W:(p + 1) * W], ps[:, 0:W])
                        nc.scalar.copy(stB[:, p * W:(p + 1) * W], ps[:, W:2 * W])
                    for bi, st in ((b0, stA), (b0 + 1, stB)):
                        dst = out_d[bi, :, g:g + 2 * half, :].rearrange(
                            "f (h p) c -> h f (p c)", h=2)
                        nc.scalar.dma_start(dst, st[:])

    nc.compile()
    return nc


def _prep_host(input, kern, mode):
    xpr = _make_xpr(input)
    wtab = _make_weight_tables(kern)
    if mode == "bf16":
        import ml_dtypes
        xpr = xpr.astype(ml_dtypes.bfloat16)
        wtab = wtab.astype(ml_dtypes.bfloat16)
    return xpr, wtab


def kernel(input, kernel):
    if "nc" not in _cache:
        _cache["nc"] = _build_program(MODE)
    nc = _cache["nc"]

    input = np.ascontiguousarray(np.asarray(input, dtype=np.float32))
    kern = np.ascontiguousarray(np.asarray(kernel, dtype=np.float32))
    xpr, wtab = _prep_host(input, kern, MODE)
    in_maps = [
        {"xpr": xpr[B_LOC * c:B_LOC * (c + 1)], "wtab": wtab}
        for c in range(N_CORES)
    ]
    res = run_bass_kernel_spmd(nc, in_maps, core_ids=list(range(N_CORES)))
    _cache["last_results"] = res
    return np.concatenate([r["out"] for r in res.results], axis=0)
